# revision 1
# baseline (speedup 1.0000x reference)
"""DeltaNet layer (scalar-gated 2-channel linear attention) on 8 trn2 cores.

Sharding: batch (4) x head-half (2) -> 8 cores. Each core computes rmsnorm,
q/k/v/beta projections for its 4 heads, the chunked delta-scan (C=128), and
a partial output projection y_part = o_part @ Wo[:, feats].T. Host unshard
sums the two partials per batch and adds the residual x + bo.

Math: the reference recurrence H_t = b_t*H_{t-1} + m_t*k_t v_t^T (2 decay
channels) with out_t = (q_t^T (H1+H2)) / max(q_t.(Z1+Z2), eps) * m_t is
chunkwise-parallelized: within a chunk A_T[j,i] = (k_j.q_i)*exp(c_i-c_j)
for j<=i, realized as (k_j*exp(-c_j)) . (q_i*exp(c_i)) with c = per-chunk
cumsum(log b). Cross-chunk state H [d,d] and Z [d] are carried (Z as an
extra column of H; both channels stacked on partitions). Validated vs
reference in numpy: l2 rel err ~3e-7.
"""
import sys

sys.path.insert(0, "/opt/trn_rl_repo")

import numpy as np

import bass_rust
import concourse.bass as bass
import concourse.mybir as mybir
import concourse.tile as tile
from concourse.bass_utils import run_bass_kernel_spmd
from concourse.masks import make_identity

FP = mybir.dt.float32
FPR = mybir.dt.float32r
BF = mybir.dt.bfloat16
AF = mybir.ActivationFunctionType
OP = mybir.AluOpType
ts = bass.ts

B, S, DM, H = 4, 2048, 512, 8
D = DM // H          # 64 head dim
HL = H // 2          # 4 heads per core
C = 128              # chunk length
NCH = S // C         # 16 chunks
KC = DM // 128       # 4 contraction chunks
NB = S // 512        # 4 position blocks of 512
BETA_MIN, BETA_MAX = 0.01, 0.995
EPS_DELTA, EPS_RMS = 1e-6, 1e-5


def _legalize_waits(nc, max_waits=1, max_updates=1):
    """This walrus accepts at most one sync wait/update per instruction.
    Move extras onto adjacent NoOp carriers (same engine, program order)."""
    n = 0
    for fn in nc.m.functions:
        for bb in fn.blocks:
            out = []
            changed = False
            for inst in bb.instructions:
                si = inst.sync_info
                if si is None:
                    out.append(inst)
                    continue
                waits = list(si.on_wait)
                updates = list(si.on_update)
                pre, post = [], []
                if len(waits) > max_waits:
                    extra, waits = waits[:-max_waits], waits[-max_waits:]
                    for w in extra:
                        n += 1
                        nop = mybir.InstNoOp(name=f"wcar_{n}", engine=inst.engine)
                        nop.sync_info = bass_rust.SyncInfo(on_wait=[w], on_update=[])
                        pre.append(nop)
                if len(updates) > max_updates:
                    updates, extra = updates[:max_updates], updates[max_updates:]
                    for u in extra:
                        n += 1
                        nop = mybir.InstNoOp(name=f"ucar_{n}", engine=inst.engine)
                        nop.sync_info = bass_rust.SyncInfo(on_wait=[], on_update=[u])
                        post.append(nop)
                if pre or post:
                    inst.sync_info = bass_rust.SyncInfo(on_wait=waits, on_update=updates)
                    changed = True
                out.extend(pre)
                out.append(inst)
                out.extend(post)
            if changed:
                bb.instructions = out
    return n


def _dram_bcast(ap, nparts):
    """Stride-0 partition broadcast AP over a DRAM source."""
    return bass.AP(tensor=ap.tensor, offset=ap.offset,
                   ap=[[0, nparts]] + [list(x) for x in ap.ap])


def build_program(b1_base: float, b2_base: float, legalize: bool = True):
    nc = bass.Bass()

    xT = nc.dram_tensor("xT", [DM, S], FPR, kind="ExternalInput")
    mask16 = nc.dram_tensor("mask16", [NCH, C], FP, kind="ExternalInput")
    wq = nc.dram_tensor("wq", [DM, HL * 2 * D], FPR, kind="ExternalInput")
    wk = nc.dram_tensor("wk", [DM, HL * 2 * D], FPR, kind="ExternalInput")
    wkn = nc.dram_tensor("wkn", [DM, HL * D], FPR, kind="ExternalInput")
    wv = nc.dram_tensor("wv", [DM, HL * D], FPR, kind="ExternalInput")
    wb = nc.dram_tensor("wb", [DM, HL], FPR, kind="ExternalInput")
    bbr = nc.dram_tensor("bbr", [HL, 1], FP, kind="ExternalInput")
    wo = nc.dram_tensor("wo", [HL * D, DM], FPR, kind="ExternalInput")
    lt_in = nc.dram_tensor("lt", [C, C], FP, kind="ExternalInput")
    ones_in = nc.dram_tensor("ones128", [128, 128], FPR, kind="ExternalInput")
    y_out = nc.dram_tensor("y", [S, DM], FP, kind="ExternalOutput")

    with tile.TileContext(nc) as tc:
        with tc.tile_pool(name="const", bufs=1) as const, \
             tc.tile_pool(name="big", bufs=1) as big, \
             tc.tile_pool(name="work", bufs=2) as work, \
             tc.tile_pool(name="psA", bufs=2, space="PSUM") as psA, \
             tc.tile_pool(name="psB", bufs=2, space="PSUM") as psB, \
             tc.tile_pool(name="psC", bufs=1, space="PSUM") as psC, \
             tc.tile_pool(name="psD", bufs=2, space="PSUM") as psD, \
             tc.tile_pool(name="work1", bufs=1) as work1, \
             tc.tile_pool(name="dram", bufs=1, space="DRAM") as dram, \
             tc.tile_pool(name="pha", bufs=1) as pha:

            # ---------------- constants ----------------
            ones128 = const.tile([128, 128], FPR)
            nc.sync.dma_start(ones128[:], ones_in[:])
            lt = const.tile([C, C], FP)
            nc.sync.dma_start(lt[:], lt_in[:])
            ident = const.tile([128, 128], FP)
            make_identity(nc, ident[:])
            mask_cj = const.tile([NCH, C], FP)
            nc.sync.dma_start(mask_cj[:], mask16[:])
            wq_t = const.tile([128, KC, HL * 2 * D], FPR)
            nc.sync.dma_start(wq_t[:], wq.rearrange("(kc p) f -> p kc f", p=128))
            wk_t = const.tile([128, KC, HL * 2 * D], FPR)
            wkn_t = const.tile([128, KC, HL * D], FPR)
            nc.sync.dma_start(wkn_t[:], wkn.rearrange("(kc p) f -> p kc f", p=128))
            nc.sync.dma_start(wk_t[:], wk.rearrange("(kc p) f -> p kc f", p=128))
            wv_t = const.tile([128, KC, HL * D], FPR)
            nc.sync.dma_start(wv_t[:], wv.rearrange("(kc p) f -> p kc f", p=128))
            wb_t = const.tile([128, KC, HL], FPR)
            nc.sync.dma_start(wb_t[:], wb.rearrange("(kc p) f -> p kc f", p=128))
            bbr_t = const.tile([HL, 1], FP)
            nc.sync.dma_start(bbr_t[:], bbr[:])
            wo_t = const.tile([128, 2, DM], FPR)
            nc.sync.dma_start(wo_t[:], wo.rearrange("(fc p) f -> p fc f", p=128))
            eps_col = const.tile([128, 1], FP)
            nc.vector.memset(eps_col[:], EPS_RMS)

            # mask in per-partition [j, c] layout
            mt_ps = psC.tile([C, NCH], FP, tag="tps")
            nc.tensor.transpose(mt_ps[:], mask_cj[:], ident[0:NCH, 0:NCH])
            mask_jc = const.tile([C, NCH], FP)
            nc.vector.tensor_copy(mask_jc[:], mt_ps[:])

            # ---------------- stage 1: rmsnorm (xt -> ht in place) ----------
            xt = pha.tile([128, KC, S], FPR, tag="xt")
            nc.sync.dma_start(xt[:], xT.rearrange("(kc p) s -> p kc s", p=128))
            ht = xt
            for nb in range(NB):
                ps = psA.tile([128, 512], FP, tag="acc512")
                for kc in range(KC):
                    x2 = work.tile([128, 512], FPR, tag="wA")
                    nc.scalar.activation(x2[:], xt[:, kc, ts(nb, 512)], AF.Square)
                    nc.tensor.matmul(ps[:], ones128[:], x2[:], start=(kc == 0),
                                     stop=(kc == KC - 1))
                lnm = work.tile([128, 512], FP, tag="wB")
                nc.scalar.activation(lnm[:], ps[:], AF.Ln, bias=eps_col[:, 0:1],
                                     scale=1.0 / DM)
                invr = work.tile([128, 512], FP, tag="wC")
                nc.scalar.activation(invr[:], lnm[:], AF.Exp, scale=-0.5)
                for kc in range(KC):
                    nc.vector.tensor_tensor(ht[:, kc, ts(nb, 512)],
                                            xt[:, kc, ts(nb, 512)], invr[:],
                                            OP.mult)

            # ------------- stage 2a: beta projection + decay cascade --------
            dbeta = dram.tile([HL, S], FP)
            for nb in range(NB):
                psb = psA.tile([HL, 512], FP, tag="acc512")
                for kc in range(KC):
                    nc.tensor.matmul(psb[:], wb_t[:, kc], ht[:, kc, ts(nb, 512)],
                                     start=(kc == 0), stop=(kc == KC - 1))
                brow = work.tile([HL, 512], FP, tag="wD")
                nc.scalar.activation(brow[:], psb[:], AF.Sigmoid,
                                     bias=bbr_t[:, 0:1])
                nc.sync.dma_start(dbeta[:, ts(nb, 512)], brow[:])
            beta_hc = work.tile([HL * NCH, C], FP, tag="wA")
            nc.sync.dma_start(beta_hc[:],
                              dbeta.rearrange("h (c j) -> (h c) j", j=C))
            bin_ = work.tile([HL * NCH, C], FP, tag="wB")
            nc.vector.tensor_scalar(bin_[:], beta_hc[:], BETA_MAX, BETA_MIN,
                                    OP.min, OP.max)
            zeros_hc = work.tile([HL * NCH, C], FP, tag="wC")
            nc.vector.memset(zeros_hc[:], 0.0)
            g, invg, invg_T = {}, {}, {}
            glast_bc = {}
            glsel = const.tile([128, HL, NCH], FP)  # rows 0:64 ch1, 64:128 ch2
            dg = {}
            for ch, bbase in ((1, b1_base), (2, b2_base)):
                bch = work.tile([HL * NCH, C], FP, tag="wD", name=f"bch{ch}")
                nc.vector.tensor_scalar(bch[:], bin_[:], bbase, BETA_MAX,
                                        OP.mult, OP.min)
                nc.vector.tensor_scalar_max(bch[:], bch[:], BETA_MIN)
                lb = work.tile([HL * NCH, C], FP, tag="wE", name=f"lb{ch}")
                nc.scalar.activation(lb[:], bch[:], AF.Ln)
                cc = work.tile([HL * NCH, C], FP, tag="wF", name=f"cc{ch}")
                nc.vector.tensor_tensor_scan(cc[:], lb[:], zeros_hc[:], 0.0,
                                             OP.add, OP.add)
                g[ch] = const.tile([HL * NCH, C], FP, tag=f"g{ch}", name=f"g{ch}")
                nc.scalar.activation(g[ch][:], cc[:], AF.Exp)
                invg[ch] = const.tile([HL * NCH, C], FP, tag=f"ig{ch}",
                                      name=f"ig{ch}")
                nc.scalar.activation(invg[ch][:], cc[:], AF.Exp, scale=-1.0)
                dg[(ch, 0)] = dram.tile([HL * NCH, C], FP, name=f"dg{ch}")
                nc.sync.dma_start(dg[(ch, 0)][:], g[ch][:])
                dg[(ch, 1)] = dram.tile([HL * NCH, C], FP, name=f"dig{ch}")
                nc.sync.dma_start(dg[(ch, 1)][:], invg[ch][:])
                # transposed forms
                pst = psC.tile([C, HL * NCH], FP, tag="tps")
                nc.tensor.transpose(pst[:], invg[ch][:], ident[0:64, 0:64])
                invg_T[ch] = const.tile([C, HL, NCH], FP, tag=f"igt{ch}",
                                        name=f"igt{ch}")
                nc.vector.tensor_copy(
                    invg_T[ch][:].rearrange("p h c -> p (h c)"), pst[:])
                psg = psC.tile([C, HL * NCH], FP, tag="tps")
                nc.tensor.transpose(psg[:], g[ch][:], ident[0:64, 0:64])
                gT = work.tile([C, HL * NCH], FP, tag="wE", name=f"gT{ch}")
                nc.vector.tensor_copy(gT[:], psg[:])
                grow = dram.tile([1, HL * NCH], FP, name=f"grow{ch}")
                nc.sync.dma_start(grow[:], gT[127:128, :])
                glast_bc[ch] = const.tile([128, HL, NCH], FP, tag=f"glb{ch}",
                                          name=f"glb{ch}")
                src = grow.rearrange("o (h c) -> (o h) c", h=HL)
                nc.sync.dma_start(glast_bc[ch][:], _dram_bcast(src, 128))
                nc.sync.dma_start(glsel[(ch - 1) * 64:ch * 64],
                                  _dram_bcast(src, 64))

            # scol[ch][j, h, ch, c] = m_j * invg_ch[j] * glast_ch (Khat mult)
            scol = const.tile([C, HL, 2, NCH], FP)
            for ch in (1, 2):
                nc.vector.tensor_tensor(
                    scol[:, :, ch - 1, :], invg_T[ch][:],
                    mask_jc[:, None, :].to_broadcast([C, HL, NCH]), OP.mult)
                nc.vector.tensor_tensor(
                    scol[:, :, ch - 1, :], scol[:, :, ch - 1, :],
                    glast_bc[ch][:], OP.mult)

            # ------------- stage 2b: natural v (V_ext) and k (Knat) ---------
            vext = big.tile([128, NCH, HL, D + 1], BF, tag="vext")
            knat = big.tile([128, NCH, HL * D], FP, tag="knat")
            nc.vector.memset(vext[:, :, :, D:D + 1], 1.0)
            for pc in range(NCH):
                psv = psA.tile([128, HL * D], FP, tag="acc512")
                for kc in range(KC):
                    nc.tensor.matmul(psv[:], ht[:, kc, ts(pc, 128)], wv_t[:, kc],
                                     start=(kc == 0), stop=(kc == KC - 1))
                nc.vector.tensor_copy(
                    vext[:, pc, :, 0:D], psv[:].rearrange("p (h d) -> p h d", h=HL))
                psk = psA.tile([128, HL * D], FP, tag="acc512")
                for kc in range(KC):
                    nc.tensor.matmul(psk[:], ht[:, kc, ts(pc, 128)], wkn_t[:, kc],
                                     start=(kc == 0), stop=(kc == KC - 1))
                ek = work.tile([128, HL * D], FP, tag="wA", name=f"ek{pc}")
                nc.scalar.activation(ek[:], psk[:], AF.Exp)
                nc.vector.tensor_scalar_min(ek[:], ek[:], 1.0)
                nc.vector.scalar_tensor_tensor(knat[:, pc], psk[:], 0.0, ek[:],
                                               OP.max, OP.add)

            # ------------- stage 2c: q/k chan-stacked projections + phi -----
            # qgs[h] rows 0:64 = phi(q_h)*g1 bcast over d-lanes, 64:128 = *g2
            # kts[h] likewise with invg. Weight rows duplicated via stride-0 AP.
            qgs, kts = {}, {}
            for h in range(HL):
                gst = work1.tile([128, NCH, C], FP, tag="gst")
                igst = work1.tile([128, NCH, C], FP, tag="igst")
                for ch in (1, 2):
                    sl = slice((ch - 1) * D, ch * D)
                    nc.sync.dma_start(
                        gst[sl], _dram_bcast(dg[(ch, 0)][ts(h, NCH), :], D))
                    nc.sync.dma_start(
                        igst[sl], _dram_bcast(dg[(ch, 1)][ts(h, NCH), :], D))
                qgs[h] = big.tile([128, S], BF, tag=f"qgs{h}", name=f"qgs{h}")
                kts[h] = big.tile([128, S], BF, tag=f"kts{h}", name=f"kts{h}")
                for (wt, dst, gtile) in ((wq_t, qgs[h], gst), (wk_t, kts[h], igst)):
                    for nb in range(NB):
                        ps = psA.tile([128, 512], FP, tag="acc512")
                        for kc in range(KC):
                            nc.tensor.matmul(ps[:], wt[:, kc, ts(h, 128)],
                                             ht[:, kc, ts(nb, 512)],
                                             start=(kc == 0), stop=(kc == KC - 1))
                        e = work.tile([128, 512], FP, tag="wB", name=f"e{h}{nb}")
                        nc.scalar.activation(e[:], ps[:], AF.Exp)
                        nc.vector.tensor_scalar_min(e[:], e[:], 1.0)
                        nc.vector.scalar_tensor_tensor(
                            dst[:, ts(nb, 512)], ps[:], 0.0, e[:], OP.max, OP.add)
                    nc.vector.tensor_tensor(
                        dst[:].rearrange("p (c j) -> p c j", j=C),
                        dst[:].rearrange("p (c j) -> p c j", j=C),
                        gtile[:], OP.mult)

            # ------------- stage 3: fused scan + out-projection -------------
            hstate = big.tile([128, HL, D + 1], FP, tag="hstate")
            nc.vector.memset(hstate[:], 0.0)
            hsb = big.tile([128, HL, D + 1], BF, tag="hsb")
            nc.vector.memset(hsb[:], 0.0)
            for pc in range(NCH):
                ps_s = psB.tile([128, HL, C], FP, tag="ps_s")
                for h in range(HL):
                    nc.tensor.matmul(ps_s[:, h], kts[h][:, ts(pc, C)],
                                     qgs[h][:, ts(pc, C)], start=True, stop=True)
                # mask m_j folded here; V_ext stays unmasked
                at = work.tile([128, HL, C], BF, tag="wD", name=f"at{pc}")
                nc.vector.scalar_tensor_tensor(
                    at[:], ps_s[:], mask_jc[:, pc:pc + 1],
                    lt[:, None, :].to_broadcast([C, HL, C]), OP.mult, OP.mult)
                khat = work.tile([128, HL, 2, D], BF, tag="wE", name=f"kh{pc}")
                nc.vector.tensor_tensor(
                    khat[:],
                    knat[:, pc].rearrange("p (h d) -> p h d", h=HL)[
                        :, :, None, :].to_broadcast([128, HL, 2, D]),
                    scol[:, :, :, pc:pc + 1].to_broadcast([128, HL, 2, D]),
                    OP.mult)
                ps_o = psD.tile([128, HL, D + 1], FP, tag="ps_o")
                ps_h = psC.tile([128, HL, D + 1], FP, tag="ps_h")
                for h in range(HL):
                    nc.tensor.matmul(ps_o[:, h], at[:, h], vext[:, pc, h],
                                     start=True, stop=False)
                    nc.tensor.matmul(ps_o[:, h], qgs[h][:, ts(pc, C)],
                                     hsb[:, h], start=False, stop=True)
                    nc.tensor.matmul(ps_h[:, h],
                                     khat[:, h].rearrange("p t d -> p (t d)"),
                                     vext[:, pc, h], start=True, stop=True)
                den = work.tile([128, HL], FP, tag="wden", name=f"den{pc}")
                nc.vector.tensor_scalar_max(den[:], ps_o[:, :, D], EPS_DELTA)
                rden = work.tile([128, HL], FP, tag="wrd", name=f"rden{pc}")
                nc.vector.reciprocal(rden[:], den[:])
                onat = work.tile([128, HL, D], FP, tag="wC", name=f"on{pc}")
                nc.vector.scalar_tensor_tensor(
                    onat[:], ps_o[:, :, 0:D], mask_jc[:, pc:pc + 1],
                    rden[:, :, None].to_broadcast([128, HL, D]),
                    OP.mult, OP.mult)
                hs2 = work.tile([128, HL, D + 1], FP, tag="wF", name=f"hs2{pc}")
                nc.vector.tensor_tensor(
                    hs2[:], hstate[:],
                    glsel[:, :, pc:pc + 1].to_broadcast([128, HL, D + 1]),
                    OP.mult)
                nc.vector.tensor_tensor(hstate[:], hs2[:], ps_h[:], OP.add)
                nc.vector.tensor_copy(hsb[:], hstate[:])
                # transpose o and accumulate the partial out-projection
                psy = psA.tile([128, DM], FP, tag="acc512")
                for fc in range(2):
                    pst = psB.tile([128, 128], FP, tag="ps_s")
                    nc.tensor.transpose(
                        pst[:],
                        onat[:].rearrange("p h d -> p (h d)")[:, ts(fc, 128)],
                        ident[:])
                    otb = work.tile([128, 128], FPR, tag="wot", name=f"ot{pc}{fc}")
                    nc.vector.tensor_copy(otb[:], pst[:])
                    nc.tensor.matmul(psy[:], otb[:], wo_t[:, fc],
                                     start=(fc == 0), stop=(fc == 1))
                ych = work.tile([128, DM], FP, tag="wA", name=f"y{pc}")
                nc.scalar.activation(ych[:], psy[:], AF.Copy)
                nc.sync.dma_start(y_out[ts(pc, C), :], ych[:])

    if legalize:
        _legalize_waits(nc)
    return nc


def _dup_heads(wt):
    # [DM, HL*D] -> [DM, (HL, 2, D)] head block repeated on both row-halves
    w = wt.reshape(DM, HL, 1, D)
    return np.ascontiguousarray(np.broadcast_to(w, (DM, HL, 2, D))
                                .reshape(DM, HL * 2 * D))


def host_prepare(inputs):
    x = np.asarray(inputs["x"], np.float32)
    mask = np.asarray(inputs["mask"], np.float32)
    Wq = np.asarray(inputs["Wq"], np.float32)
    Wk = np.asarray(inputs["Wk"], np.float32)
    Wv = np.asarray(inputs["Wv"], np.float32)
    Wb = np.asarray(inputs["Wb"], np.float32)
    bb = np.asarray(inputs["bb"], np.float32)
    Wo = np.asarray(inputs["Wo"], np.float32)
    bo = np.asarray(inputs["bo"], np.float32)
    ln_w = np.asarray(inputs["ln_w"], np.float32)
    rec = np.asarray(inputs["recency"], np.float32)
    b1b = float(np.clip(1.0 / (1.0 + np.exp(-float(inputs["base_beta_1"]))),
                        BETA_MIN, BETA_MAX))
    b2b = float(np.clip(1.0 / (1.0 + np.exp(-float(inputs["base_beta_2"]))),
                        BETA_MIN, BETA_MAX))
    lt = np.triu(np.ones((C, C), np.float32))  # [j, i]: keep j<=i
    ones128 = np.ones((128, 128), np.float32)
    in_maps = []
    for core in range(8):
        b, g2 = divmod(core, 2)
        hsl = slice(g2 * HL, (g2 + 1) * HL)
        fsl = slice(g2 * HL * D, (g2 + 1) * HL * D)
        in_maps.append({
            "xT": np.ascontiguousarray(x[b].T),
            "mask16": np.ascontiguousarray(mask[b].reshape(NCH, C)),
            "wq": _dup_heads((Wq[fsl] * ln_w[None, :]).T),
            "wk": _dup_heads((Wk[fsl] * ln_w[None, :]).T),
            "wkn": np.ascontiguousarray((Wk[fsl] * ln_w[None, :]).T),
            "wv": np.ascontiguousarray((Wv[fsl] * ln_w[None, :]).T),
            "wb": np.ascontiguousarray((Wb[hsl] * ln_w[None, :]).T),
            "bbr": np.ascontiguousarray((bb[hsl] + rec[hsl])[:, None]),
            "wo": np.ascontiguousarray(Wo[:, fsl].T),
            "lt": lt,
            "ones128": ones128,
        })
    return in_maps, dict(x=x, bo=bo, b1b=b1b, b2b=b2b)


_CACHE = {}


def _get_program(b1b, b2b):
    key = (b1b, b2b)
    if key not in _CACHE:
        _CACHE[key] = build_program(b1b, b2b)
    return _CACHE[key]


def kernel(**inputs) -> np.ndarray:
    in_maps, prep = host_prepare(inputs)
    nc = _get_program(prep["b1b"], prep["b2b"])
    res = run_bass_kernel_spmd(nc, in_maps, core_ids=list(range(8)))
    x, bo = prep["x"], prep["bo"]
    out = np.empty((B, S, DM), np.float32)
    for b in range(B):
        out[b] = x[b] + res.results[2 * b]["y"] + res.results[2 * b + 1]["y"] + bo
    return out



# revision 6
# speedup vs baseline: 1.0331x; 1.0331x over previous
"""DeltaNet layer (scalar-gated 2-channel linear attention) on 8 trn2 cores.

Sharding: batch (4) x head-half (2) -> 8 cores. Each core computes rmsnorm,
q/k/v/beta projections for its 4 heads, the chunked delta-scan (C=128), and
a partial output projection y_part = o_part @ Wo[:, feats].T. Host unshard
sums the two partials per batch and adds the residual x + bo.

Math: the reference recurrence H_t = b_t*H_{t-1} + m_t*k_t v_t^T (2 decay
channels) with out_t = (q_t^T (H1+H2)) / max(q_t.(Z1+Z2), eps) * m_t is
chunkwise-parallelized: within a chunk A_T[j,i] = (k_j.q_i)*exp(c_i-c_j)
for j<=i, realized as (k_j*exp(-c_j)) . (q_i*exp(c_i)) with c = per-chunk
cumsum(log b). Cross-chunk state H [d,d] and Z [d] are carried (Z as an
extra column of H; both channels stacked on partitions). Validated vs
reference in numpy: l2 rel err ~3e-7.
"""
import sys

sys.path.insert(0, "/opt/trn_rl_repo")

import numpy as np

import bass_rust
import concourse.bass as bass
import concourse.mybir as mybir
import concourse.tile as tile
from concourse.bass_utils import run_bass_kernel_spmd
from concourse.masks import make_identity

FP = mybir.dt.float32
FPR = mybir.dt.float32r
BF = mybir.dt.bfloat16
AF = mybir.ActivationFunctionType
OP = mybir.AluOpType
ts = bass.ts

B, S, DM, H = 4, 2048, 512, 8
D = DM // H          # 64 head dim
HL = H // 2          # 4 heads per core
C = 128              # chunk length
NCH = S // C         # 16 chunks
KC = DM // 128       # 4 contraction chunks
NB = S // 512        # 4 position blocks of 512
BETA_MIN, BETA_MAX = 0.01, 0.995
EPS_DELTA, EPS_RMS = 1e-6, 1e-5


def _legalize_waits(nc, max_waits=1, max_updates=1):
    """This walrus accepts at most one sync wait/update per instruction.
    Move extras onto adjacent NoOp carriers (same engine, program order)."""
    n = 0
    for fn in nc.m.functions:
        for bb in fn.blocks:
            out = []
            changed = False
            for inst in bb.instructions:
                si = inst.sync_info
                if si is None:
                    out.append(inst)
                    continue
                waits = list(si.on_wait)
                updates = list(si.on_update)
                pre, post = [], []
                if len(waits) > max_waits:
                    extra, waits = waits[:-max_waits], waits[-max_waits:]
                    for w in extra:
                        n += 1
                        nop = mybir.InstNoOp(name=f"wcar_{n}", engine=inst.engine)
                        nop.sync_info = bass_rust.SyncInfo(on_wait=[w], on_update=[])
                        pre.append(nop)
                if len(updates) > max_updates:
                    updates, extra = updates[:max_updates], updates[max_updates:]
                    for u in extra:
                        n += 1
                        nop = mybir.InstNoOp(name=f"ucar_{n}", engine=inst.engine)
                        nop.sync_info = bass_rust.SyncInfo(on_wait=[], on_update=[u])
                        post.append(nop)
                if pre or post:
                    inst.sync_info = bass_rust.SyncInfo(on_wait=waits, on_update=updates)
                    changed = True
                out.extend(pre)
                out.append(inst)
                out.extend(post)
            if changed:
                bb.instructions = out
    return n


def _dram_bcast(ap, nparts):
    """Stride-0 partition broadcast AP over a DRAM source."""
    return bass.AP(tensor=ap.tensor, offset=ap.offset,
                   ap=[[0, nparts]] + [list(x) for x in ap.ap])


def build_program(b1_base: float, b2_base: float, legalize: bool = True):
    nc = bass.Bass()

    xT = nc.dram_tensor("xT", [DM, S], FPR, kind="ExternalInput")
    mask16 = nc.dram_tensor("mask16", [NCH, C], FP, kind="ExternalInput")
    wq = nc.dram_tensor("wq", [DM, HL * 2 * D], FPR, kind="ExternalInput")
    wk = nc.dram_tensor("wk", [DM, HL * 2 * D], FPR, kind="ExternalInput")
    wkn = nc.dram_tensor("wkn", [DM, HL * D], FPR, kind="ExternalInput")
    wv = nc.dram_tensor("wv", [DM, HL * D], FPR, kind="ExternalInput")
    wb = nc.dram_tensor("wb", [DM, HL], FPR, kind="ExternalInput")
    bbr = nc.dram_tensor("bbr", [HL, 1], FP, kind="ExternalInput")
    wo = nc.dram_tensor("wo", [HL * D, DM], FPR, kind="ExternalInput")
    lt_in = nc.dram_tensor("lt", [C, C], FP, kind="ExternalInput")
    ones_in = nc.dram_tensor("ones128", [128, 128], FPR, kind="ExternalInput")
    y_out = nc.dram_tensor("y", [S, DM], FP, kind="ExternalOutput")

    with tile.TileContext(nc) as tc:
        with tc.tile_pool(name="const", bufs=1) as const, \
             tc.tile_pool(name="big", bufs=1) as big, \
             tc.tile_pool(name="work", bufs=2) as work, \
             tc.tile_pool(name="psA", bufs=2, space="PSUM") as psA, \
             tc.tile_pool(name="psB", bufs=2, space="PSUM") as psB, \
             tc.tile_pool(name="psC", bufs=1, space="PSUM") as psC, \
             tc.tile_pool(name="psD", bufs=2, space="PSUM") as psD, \
             tc.tile_pool(name="work1", bufs=2) as work1, \
             tc.tile_pool(name="dram", bufs=1, space="DRAM") as dram, \
             tc.tile_pool(name="pha", bufs=1) as pha:

            # ---------------- constants ----------------
            ones128 = const.tile([128, 128], FPR)
            nc.sync.dma_start(ones128[:], ones_in[:])
            lt = const.tile([C, C], FP)
            nc.sync.dma_start(lt[:], lt_in[:])
            ident = const.tile([128, 128], FP)
            make_identity(nc, ident[:])
            mask_cj = const.tile([NCH, C], FP)
            nc.sync.dma_start(mask_cj[:], mask16[:])
            wq_t = const.tile([128, KC, HL * 2 * D], FPR)
            nc.sync.dma_start(wq_t[:], wq.rearrange("(kc p) f -> p kc f", p=128))
            wk_t = const.tile([128, KC, HL * 2 * D], FPR)
            wkn_t = const.tile([128, KC, HL * D], FPR)
            nc.sync.dma_start(wkn_t[:], wkn.rearrange("(kc p) f -> p kc f", p=128))
            nc.sync.dma_start(wk_t[:], wk.rearrange("(kc p) f -> p kc f", p=128))
            wv_t = const.tile([128, KC, HL * D], FPR)
            nc.sync.dma_start(wv_t[:], wv.rearrange("(kc p) f -> p kc f", p=128))
            wb_t = const.tile([128, KC, HL], FPR)
            nc.sync.dma_start(wb_t[:], wb.rearrange("(kc p) f -> p kc f", p=128))
            bbr_t = const.tile([HL, 1], FP)
            nc.sync.dma_start(bbr_t[:], bbr[:])
            wo_t = const.tile([128, 2, DM], FPR)
            nc.sync.dma_start(wo_t[:], wo.rearrange("(fc p) f -> p fc f", p=128))
            eps_col = const.tile([128, 1], FP)
            nc.vector.memset(eps_col[:], EPS_RMS)

            # mask in per-partition [j, c] layout
            mt_ps = psC.tile([C, NCH], FP, tag="tps")
            nc.tensor.transpose(mt_ps[:], mask_cj[:], ident[0:NCH, 0:NCH])
            mask_jc = const.tile([C, NCH], FP)
            nc.vector.tensor_copy(mask_jc[:], mt_ps[:])

            # ---------------- stage 1: rmsnorm (xt -> ht in place) ----------
            xt = pha.tile([128, KC, S], FPR, tag="xt")
            xview = xT.rearrange("(kc p) s -> p kc s", p=128)
            for nb in range(NB):
                nc.sync.dma_start(xt[:, :, ts(nb, 512)],
                                  xview[:, :, ts(nb, 512)])
            ht = xt
            for nb in range(NB):
                ps = psA.tile([128, 512], FP, tag="acc512")
                for kc in range(KC):
                    x2 = work.tile([128, 512], FPR, tag="wA")
                    nc.scalar.activation(x2[:], xt[:, kc, ts(nb, 512)], AF.Square)
                    nc.tensor.matmul(ps[:], ones128[:], x2[:], start=(kc == 0),
                                     stop=(kc == KC - 1))
                lnm = work.tile([128, 512], FP, tag="wB")
                nc.scalar.activation(lnm[:], ps[:], AF.Ln, bias=eps_col[:, 0:1],
                                     scale=1.0 / DM)
                invr = work.tile([128, 512], FP, tag="wC")
                nc.scalar.activation(invr[:], lnm[:], AF.Exp, scale=-0.5)
                for kc in range(KC):
                    nc.vector.tensor_tensor(ht[:, kc, ts(nb, 512)],
                                            xt[:, kc, ts(nb, 512)], invr[:],
                                            OP.mult)

            # ------------- stage 2a: beta projection + decay cascade --------
            dbeta = dram.tile([HL, S], FP)
            for nb in range(NB):
                psb = psA.tile([HL, 512], FP, tag="acc512")
                for kc in range(KC):
                    nc.tensor.matmul(psb[:], wb_t[:, kc], ht[:, kc, ts(nb, 512)],
                                     start=(kc == 0), stop=(kc == KC - 1))
                brow = work.tile([HL, 512], FP, tag="wD")
                nc.scalar.activation(brow[:], psb[:], AF.Sigmoid,
                                     bias=bbr_t[:, 0:1])
                nc.sync.dma_start(dbeta[:, ts(nb, 512)], brow[:])
            beta_hc = work.tile([HL * NCH, C], FP, tag="wA")
            nc.sync.dma_start(beta_hc[:],
                              dbeta.rearrange("h (c j) -> (h c) j", j=C))
            bin_ = work.tile([HL * NCH, C], FP, tag="wB")
            nc.vector.tensor_scalar(bin_[:], beta_hc[:], BETA_MAX, BETA_MIN,
                                    OP.min, OP.max)
            zeros_hc = work.tile([HL * NCH, C], FP, tag="wC")
            nc.vector.memset(zeros_hc[:], 0.0)
            g, invg, invg_T = {}, {}, {}
            glast_bc = {}
            glsel = const.tile([128, HL, NCH], FP)  # rows 0:64 ch1, 64:128 ch2
            dg = {}
            for ch, bbase in ((1, b1_base), (2, b2_base)):
                bch = work.tile([HL * NCH, C], FP, tag="wD", name=f"bch{ch}")
                nc.vector.tensor_scalar(bch[:], bin_[:], bbase, BETA_MAX,
                                        OP.mult, OP.min)
                nc.vector.tensor_scalar_max(bch[:], bch[:], BETA_MIN)
                lb = work.tile([HL * NCH, C], FP, tag="wE", name=f"lb{ch}")
                nc.scalar.activation(lb[:], bch[:], AF.Ln)
                cc = work.tile([HL * NCH, C], FP, tag="wF", name=f"cc{ch}")
                nc.vector.tensor_tensor_scan(cc[:], lb[:], zeros_hc[:], 0.0,
                                             OP.add, OP.add)
                g[ch] = const.tile([HL * NCH, C], FP, tag=f"g{ch}", name=f"g{ch}")
                nc.scalar.activation(g[ch][:], cc[:], AF.Exp)
                invg[ch] = const.tile([HL * NCH, C], FP, tag=f"ig{ch}",
                                      name=f"ig{ch}")
                nc.scalar.activation(invg[ch][:], cc[:], AF.Exp, scale=-1.0)
                dg[(ch, 0)] = dram.tile([HL * NCH, C], FP, name=f"dg{ch}")
                nc.sync.dma_start(dg[(ch, 0)][:], g[ch][:])
                dg[(ch, 1)] = dram.tile([HL * NCH, C], FP, name=f"dig{ch}")
                nc.sync.dma_start(dg[(ch, 1)][:], invg[ch][:])
                # transposed forms
                pst = psC.tile([C, HL * NCH], FP, tag="tps")
                nc.tensor.transpose(pst[:], invg[ch][:], ident[0:64, 0:64])
                invg_T[ch] = const.tile([C, HL, NCH], FP, tag=f"igt{ch}",
                                        name=f"igt{ch}")
                nc.vector.tensor_copy(
                    invg_T[ch][:].rearrange("p h c -> p (h c)"), pst[:])
                psg = psC.tile([C, HL * NCH], FP, tag="tps")
                nc.tensor.transpose(psg[:], g[ch][:], ident[0:64, 0:64])
                gT = work.tile([C, HL * NCH], FP, tag="wE", name=f"gT{ch}")
                nc.vector.tensor_copy(gT[:], psg[:])
                grow = dram.tile([1, HL * NCH], FP, name=f"grow{ch}")
                nc.sync.dma_start(grow[:], gT[127:128, :])
                glast_bc[ch] = const.tile([128, HL, NCH], FP, tag=f"glb{ch}",
                                          name=f"glb{ch}")
                src = grow.rearrange("o (h c) -> (o h) c", h=HL)
                nc.sync.dma_start(glast_bc[ch][:], _dram_bcast(src, 128))
                nc.sync.dma_start(glsel[(ch - 1) * 64:ch * 64],
                                  _dram_bcast(src, 64))

            # scol[ch][j, h, ch, c] = m_j * invg_ch[j] * glast_ch (Khat mult)
            scol = const.tile([C, HL, 2, NCH], FP)
            for ch in (1, 2):
                nc.vector.tensor_tensor(
                    scol[:, :, ch - 1, :], invg_T[ch][:],
                    mask_jc[:, None, :].to_broadcast([C, HL, NCH]), OP.mult)
                nc.vector.tensor_tensor(
                    scol[:, :, ch - 1, :], scol[:, :, ch - 1, :],
                    glast_bc[ch][:], OP.mult)

            # ------------- stage 2b: natural v (V_ext) and k (Knat) ---------
            vext = big.tile([128, NCH, HL, D + 1], BF, tag="vext")
            knat = big.tile([128, NCH, HL * D], FP, tag="knat")
            nc.vector.memset(vext[:, :, :, D:D + 1], 1.0)
            for pc in range(NCH):
                psv = psA.tile([128, HL * D], FP, tag="acc512")
                for kc in range(KC):
                    nc.tensor.matmul(psv[:], ht[:, kc, ts(pc, 128)], wv_t[:, kc],
                                     start=(kc == 0), stop=(kc == KC - 1))
                nc.vector.tensor_copy(
                    vext[:, pc, :, 0:D], psv[:].rearrange("p (h d) -> p h d", h=HL))
                psk = psA.tile([128, HL * D], FP, tag="acc512")
                for kc in range(KC):
                    nc.tensor.matmul(psk[:], ht[:, kc, ts(pc, 128)], wkn_t[:, kc],
                                     start=(kc == 0), stop=(kc == KC - 1))
                ek = work.tile([128, HL * D], FP, tag="wA", name=f"ek{pc}")
                nc.scalar.activation(ek[:], psk[:], AF.Exp)
                nc.vector.tensor_scalar_min(ek[:], ek[:], 1.0)
                nc.vector.scalar_tensor_tensor(knat[:, pc], psk[:], 0.0, ek[:],
                                               OP.max, OP.add)

            # ------------- stage 2c: q/k chan-stacked projections + phi -----
            # qgs[h] rows 0:64 = phi(q_h)*g1 bcast over d-lanes, 64:128 = *g2
            # kts[h] likewise with invg. Weight rows duplicated via stride-0 AP.
            qgs, kts = {}, {}
            for h in range(HL):
                gst = work1.tile([128, NCH, C], FP, tag="gst")
                igst = work1.tile([128, NCH, C], FP, tag="igst")
                for ch in (1, 2):
                    sl = slice((ch - 1) * D, ch * D)
                    nc.sync.dma_start(
                        gst[sl], _dram_bcast(dg[(ch, 0)][ts(h, NCH), :], D))
                    nc.sync.dma_start(
                        igst[sl], _dram_bcast(dg[(ch, 1)][ts(h, NCH), :], D))
                qgs[h] = big.tile([128, S], BF, tag=f"qgs{h}", name=f"qgs{h}")
                kts[h] = big.tile([128, S], BF, tag=f"kts{h}", name=f"kts{h}")
                for (wt, dst, gtile) in ((wq_t, qgs[h], gst), (wk_t, kts[h], igst)):
                    for nb in range(NB):
                        ps = psA.tile([128, 512], FP, tag="acc512")
                        for kc in range(KC):
                            nc.tensor.matmul(ps[:], wt[:, kc, ts(h, 128)],
                                             ht[:, kc, ts(nb, 512)],
                                             start=(kc == 0), stop=(kc == KC - 1))
                        e = work.tile([128, 512], FP, tag="wB", name=f"e{h}{nb}")
                        nc.scalar.activation(e[:], ps[:], AF.Exp)
                        nc.vector.tensor_scalar_min(e[:], e[:], 1.0)
                        nc.vector.scalar_tensor_tensor(
                            dst[:, ts(nb, 512)], ps[:], 0.0, e[:], OP.max, OP.add)
                    nc.vector.tensor_tensor(
                        dst[:].rearrange("p (c j) -> p c j", j=C),
                        dst[:].rearrange("p (c j) -> p c j", j=C),
                        gtile[:], OP.mult)

            # ------------- stage 3: fused scan + out-projection -------------
            hstate = big.tile([128, HL, D + 1], FP, tag="hstate")
            nc.vector.memset(hstate[:], 0.0)
            hsb = big.tile([128, HL, D + 1], BF, tag="hsb")
            nc.vector.memset(hsb[:], 0.0)
            for pc in range(NCH):
                ps_s = psB.tile([128, HL, C], FP, tag="ps_s")
                for h in range(HL):
                    nc.tensor.matmul(ps_s[:, h], kts[h][:, ts(pc, C)],
                                     qgs[h][:, ts(pc, C)], start=True, stop=True)
                # mask m_j folded here; V_ext stays unmasked
                at = work.tile([128, HL, C], BF, tag="wD", name=f"at{pc}")
                nc.vector.scalar_tensor_tensor(
                    at[:], ps_s[:], mask_jc[:, pc:pc + 1],
                    lt[:, None, :].to_broadcast([C, HL, C]), OP.mult, OP.mult)
                khat = work.tile([128, HL, 2, D], BF, tag="wE", name=f"kh{pc}")
                nc.vector.tensor_tensor(
                    khat[:],
                    knat[:, pc].rearrange("p (h d) -> p h d", h=HL)[
                        :, :, None, :].to_broadcast([128, HL, 2, D]),
                    scol[:, :, :, pc:pc + 1].to_broadcast([128, HL, 2, D]),
                    OP.mult)
                ps_o = psD.tile([128, HL, D + 1], FP, tag="ps_o")
                ps_h = psC.tile([128, HL, D + 1], FP, tag="ps_h")
                for h in range(HL):
                    nc.tensor.matmul(ps_o[:, h], at[:, h], vext[:, pc, h],
                                     start=True, stop=False)
                    nc.tensor.matmul(ps_o[:, h], qgs[h][:, ts(pc, C)],
                                     hsb[:, h], start=False, stop=True)
                    nc.tensor.matmul(ps_h[:, h],
                                     khat[:, h].rearrange("p t d -> p (t d)"),
                                     vext[:, pc, h], start=True, stop=True)
                den = work.tile([128, HL], FP, tag="wden", name=f"den{pc}")
                nc.vector.tensor_scalar_max(den[:], ps_o[:, :, D], EPS_DELTA)
                rden = work.tile([128, HL], FP, tag="wrd", name=f"rden{pc}")
                nc.vector.reciprocal(rden[:], den[:])
                onat = work.tile([128, HL, D], FP, tag="wC", name=f"on{pc}")
                nc.vector.scalar_tensor_tensor(
                    onat[:], ps_o[:, :, 0:D], mask_jc[:, pc:pc + 1],
                    rden[:, :, None].to_broadcast([128, HL, D]),
                    OP.mult, OP.mult)
                hs2 = work.tile([128, HL, D + 1], FP, tag="wF", name=f"hs2{pc}")
                nc.vector.tensor_tensor(
                    hs2[:], hstate[:],
                    glsel[:, :, pc:pc + 1].to_broadcast([128, HL, D + 1]),
                    OP.mult)
                nc.vector.tensor_tensor(hstate[:], hs2[:], ps_h[:], OP.add)
                nc.vector.tensor_copy(hsb[:], hstate[:])
                # transpose o and accumulate the partial out-projection
                psy = psA.tile([128, DM], FP, tag="acc512")
                for fc in range(2):
                    pst = psB.tile([128, 128], FP, tag="ps_s")
                    nc.tensor.transpose(
                        pst[:],
                        onat[:].rearrange("p h d -> p (h d)")[:, ts(fc, 128)],
                        ident[:])
                    otb = work.tile([128, 128], FPR, tag="wot", name=f"ot{pc}{fc}")
                    nc.vector.tensor_copy(otb[:], pst[:])
                    nc.tensor.matmul(psy[:], otb[:], wo_t[:, fc],
                                     start=(fc == 0), stop=(fc == 1))
                ych = work.tile([128, DM], FP, tag="wA", name=f"y{pc}")
                nc.scalar.activation(ych[:], psy[:], AF.Copy)
                nc.sync.dma_start(y_out[ts(pc, C), :], ych[:])

    if legalize:
        _legalize_waits(nc)
    return nc


def _dup_heads(wt):
    # [DM, HL*D] -> [DM, (HL, 2, D)] head block repeated on both row-halves
    w = wt.reshape(DM, HL, 1, D)
    return np.ascontiguousarray(np.broadcast_to(w, (DM, HL, 2, D))
                                .reshape(DM, HL * 2 * D))


def host_prepare(inputs):
    x = np.asarray(inputs["x"], np.float32)
    mask = np.asarray(inputs["mask"], np.float32)
    Wq = np.asarray(inputs["Wq"], np.float32)
    Wk = np.asarray(inputs["Wk"], np.float32)
    Wv = np.asarray(inputs["Wv"], np.float32)
    Wb = np.asarray(inputs["Wb"], np.float32)
    bb = np.asarray(inputs["bb"], np.float32)
    Wo = np.asarray(inputs["Wo"], np.float32)
    bo = np.asarray(inputs["bo"], np.float32)
    ln_w = np.asarray(inputs["ln_w"], np.float32)
    rec = np.asarray(inputs["recency"], np.float32)
    b1b = float(np.clip(1.0 / (1.0 + np.exp(-float(inputs["base_beta_1"]))),
                        BETA_MIN, BETA_MAX))
    b2b = float(np.clip(1.0 / (1.0 + np.exp(-float(inputs["base_beta_2"]))),
                        BETA_MIN, BETA_MAX))
    lt = np.triu(np.ones((C, C), np.float32))  # [j, i]: keep j<=i
    ones128 = np.ones((128, 128), np.float32)
    in_maps = []
    for core in range(8):
        b, g2 = divmod(core, 2)
        hsl = slice(g2 * HL, (g2 + 1) * HL)
        fsl = slice(g2 * HL * D, (g2 + 1) * HL * D)
        in_maps.append({
            "xT": np.ascontiguousarray(x[b].T),
            "mask16": np.ascontiguousarray(mask[b].reshape(NCH, C)),
            "wq": _dup_heads((Wq[fsl] * ln_w[None, :]).T),
            "wk": _dup_heads((Wk[fsl] * ln_w[None, :]).T),
            "wkn": np.ascontiguousarray((Wk[fsl] * ln_w[None, :]).T),
            "wv": np.ascontiguousarray((Wv[fsl] * ln_w[None, :]).T),
            "wb": np.ascontiguousarray((Wb[hsl] * ln_w[None, :]).T),
            "bbr": np.ascontiguousarray((bb[hsl] + rec[hsl])[:, None]),
            "wo": np.ascontiguousarray(Wo[:, fsl].T),
            "lt": lt,
            "ones128": ones128,
        })
    return in_maps, dict(x=x, bo=bo, b1b=b1b, b2b=b2b)


_CACHE = {}


def _get_program(b1b, b2b):
    key = (b1b, b2b)
    if key not in _CACHE:
        _CACHE[key] = build_program(b1b, b2b)
    return _CACHE[key]


def kernel(**inputs) -> np.ndarray:
    in_maps, prep = host_prepare(inputs)
    nc = _get_program(prep["b1b"], prep["b2b"])
    res = run_bass_kernel_spmd(nc, in_maps, core_ids=list(range(8)))
    x, bo = prep["x"], prep["bo"]
    out = np.empty((B, S, DM), np.float32)
    for b in range(B):
        out[b] = x[b] + res.results[2 * b]["y"] + res.results[2 * b + 1]["y"] + bo
    return out

